# revision 10
# baseline (speedup 1.0000x reference)
"""Maxish pooling kernel for Trainium2 (8 NeuronCores, data-parallel).

Reference math (per row of length N):
    m  = max(x)
    rt = (x - m) / (m + 1e-8)
    pos = m * sum(exp(2*rt)) / sum(exp(rt))     # s == 1 softmax identity
    neg = m                                      # softmax sums to 1
    out = m > 0 ? pos : (m < 0 ? m : 0)

Layout: rows on partitions (128/tile), N=256 on the free axis.

Engine split (fast path, s == 1), x cast to bf16 during the DMA load
(SWDGE) so 16-bit DVE modes engage; rel-err budget (2e-2) dwarfs the
bf16 rounding:
  - DVE: pairwise tensor_tensor max folds at 2x rate (bf16), then a
    1x tensor_reduce on the folded remainder; per-tile bn_stats gives
    both sum(u) and sum(u^2).
  - ACT: per-tile exp (per-partition scale/bias APs). Optional z-path
    (z_tiles) and ACT-sum chunks (as_chunks) rebalance ACT vs DVE.
  - GpSimd: SWDGE descriptor generation + optional small/post ops
    (Pool TensorTensor only supports add/mult/sub, so no max folding
    there).
"""

import numpy as np

P = 128
N = 256
SMALL = 1e-8


def _build(n_rows: int, s: float, G: int = 16, fold_k: int = 3,
           z_dve: int = 0, z_gps: int = 8, as_chunks: int = 4,
           x_bufs: int = 3, u_bufs: int = 2, f_bufs: int = 2,
           xdt: str = "bf16", smalls_gpsimd: bool = False,
           post_gpsimd: bool = False, exact_recip: bool = False):
    from concourse import bacc, mybir
    from concourse import masks
    from concourse.tile import TileContext

    f32 = mybir.dt.float32
    bf16 = mybir.dt.bfloat16
    dt_x = bf16 if xdt == "bf16" else f32
    Act = mybir.ActivationFunctionType
    Alu = mybir.AluOpType
    Ax = mybir.AxisListType

    assert n_rows % (P * G) == 0
    T = n_rows // P          # tiles of [128, N]
    C = T // G               # chunks of G tiles
    fast = (s == 1.0)
    assert 0 <= fold_k <= 4 and N % (1 << fold_k) == 0
    assert 0 <= z_dve + z_gps <= G
    assert 0 <= as_chunks <= C

    nc = bacc.Bacc("TRN2", target_bir_lowering=False, debug=False,
                   num_devices=8)
    x_d = nc.declare_dram_parameter("x", [n_rows, N], f32, isOutput=False)
    out_d = nc.declare_dram_parameter("out", [n_rows], f32, isOutput=True)

    def recip(dst, src):
        if exact_recip:
            nc.vector.reciprocal(dst, src)
        else:
            nc.vector.reciprocal_approx_fast(dst, src)

    with TileContext(nc) as tc:
        with (
            tc.tile_pool(name="xp", bufs=x_bufs) as xp,
            tc.tile_pool(name="up", bufs=u_bufs) as up,
            tc.tile_pool(name="fold", bufs=f_bufs) as fp,
            tc.tile_pool(name="stat", bufs=1) as statp,
            tc.tile_pool(name="consts", bufs=4) as cpool,
            tc.tile_pool(name="psum", bufs=2, space="PSUM") as psp,
        ):
            M = statp.tile([P, T], f32, tag="M")       # per-row max
            S1 = statp.tile([P, T], f32, tag="S1")     # sum exp(2 rt)
            S2 = statp.tile([P, T], f32, tag="S2")     # sum exp(rt)
            R = statp.tile([P, T], f32, tag="R")       # final per-row result
            RT = statp.tile([P, T], f32, tag="RT")     # transposed result
            MK = statp.tile([P, T], mybir.dt.uint8, tag="MK")  # m>0 mask

            ident = statp.tile([P, P], f32, tag="ident")
            masks.make_identity(nc, ident[:])

            sm = nc.gpsimd if smalls_gpsimd else nc.vector
            pm = nc.gpsimd if post_gpsimd else nc.vector

            for c in range(C):
                xt = xp.tile([P, G * N], dt_x, tag="x")
                src = x_d[c * G * P:(c + 1) * G * P, :].rearrange(
                    "(g p) n -> p g n", p=P)
                xdma = nc.gpsimd if dt_x == bf16 else nc.sync
                xdma.dma_start(
                    out=xt[:].rearrange("p (g n) -> p g n", n=N), in_=src)

                # DVE pairwise max folds (2x rate on bf16), then reduce
                red_in = xt[:].rearrange("p (g n) -> p g n", n=N)
                w = N
                for k in range(fold_k):
                    w //= 2
                    ft = fp.tile([P, G * w], dt_x, tag=f"f{k}")
                    f3 = ft[:].rearrange("p (g n) -> p g n", n=w)
                    nc.vector.tensor_tensor(
                        out=f3, in0=red_in[:, :, 0:w],
                        in1=red_in[:, :, w:2 * w], op=Alu.max)
                    red_in = f3
                mg = M[:, c * G:(c + 1) * G]
                nc.vector.tensor_reduce(out=mg, in_=red_in, axis=Ax.X,
                                        op=Alu.max)

                # per-chunk consts in a versioned pool tile so ACT's reads
                # of chunk c don't serialize against DVE writing chunk c+1
                cb = cpool.tile([P, 6 * G], f32, tag="cb")
                rg = cb[:, 0:G]
                bg = cb[:, G:2 * G]
                # rg = 1 / (m + eps), clamped to >= 0 so the exponent
                # r*(x-m) stays <= 0 (m<0 rows are masked later; without
                # the clamp they can overflow exp)
                sm.tensor_scalar_add(rg, mg, SMALL)
                recip(rg, rg)
                sm.tensor_scalar_max(rg, rg, 0.0)
                # bg = (m * -1) * rg = -m/(m+eps)
                nc.vector.scalar_tensor_tensor(
                    out=bg, in0=mg, scalar=-1.0, in1=rg,
                    op0=Alu.mult, op1=Alu.mult)

                if fast:
                    act_sums = c >= C - as_chunks
                    # as-chunks take their sums from per-tile ACT accum, so
                    # they use no z-path (z tiles would have no sums)
                    zc = 0 if act_sums else z_dve + z_gps
                    ut = up.tile([P, G * N], dt_x, tag="u")
                    # z-path tiles: affine z=r*x+b on DVE/GPS, one big exp
                    for g in range(zc):
                        fs = slice(g * N, (g + 1) * N)
                        zeng = nc.vector if g < z_dve else nc.gpsimd
                        zeng.tensor_scalar(
                            out=ut[:, fs], in0=xt[:, fs],
                            scalar1=rg[:, g:g + 1], scalar2=bg[:, g:g + 1],
                            op0=Alu.mult, op1=Alu.add)
                    if zc:
                        zs = slice(0, zc * N)
                        nc.scalar.activation(out=ut[:, zs], in_=ut[:, zs],
                                             func=Act.Exp)
                    # plain tiles: per-tile exp with per-partition scale/bias
                    for g in range(zc, G):
                        fs = slice(g * N, (g + 1) * N)
                        j = c * G + g
                        if act_sums:
                            nc.scalar.activation(
                                out=ut[:, fs], in_=xt[:, fs], func=Act.Exp,
                                scale=rg[:, g:g + 1], bias=bg[:, g:g + 1],
                                accum_out=S2[:, j:j + 1])
                            nc.scalar.activation(
                                out=ut[:, fs], in_=ut[:, fs], func=Act.Square,
                                accum_out=S1[:, j:j + 1])
                        else:
                            nc.scalar.activation(
                                out=ut[:, fs], in_=xt[:, fs], func=Act.Exp,
                                scale=rg[:, g:g + 1], bias=bg[:, g:g + 1])

                    if not act_sums:
                        # both sums via per-tile bn_stats over u:
                        # 6-tuple [n_e, mu_e, M2_e, n_o, mu_o, M2_o];
                        # S2 = 128*(mu_e+mu_o);
                        # S1 = M2_e+M2_o+128*(mu_e^2+mu_o^2)
                        bst = cpool.tile([P, G * 6], f32, tag="bst")
                        for g in range(G):
                            nc.vector.bn_stats(
                                out=bst[:, g * 6:(g + 1) * 6],
                                in_=ut[:, g * N:(g + 1) * N])
                        bsg = bst[:].rearrange("p (g s) -> p s g", s=6)
                        mu_e, m2_e = bsg[:, 1], bsg[:, 2]
                        mu_o, m2_o = bsg[:, 4], bsg[:, 5]
                        s2c = S2[:, c * G:(c + 1) * G]
                        s1c = S1[:, c * G:(c + 1) * G]
                        t1 = cb[:, 2 * G:3 * G]
                        t2 = cb[:, 3 * G:4 * G]
                        t3 = cb[:, 4 * G:5 * G]
                        t4 = cb[:, 5 * G:6 * G]
                        half = float(N // 2)
                        pm.tensor_tensor(t1, mu_e, mu_o, op=Alu.add)
                        nc.vector.tensor_scalar_mul(s2c, t1, half)
                        pm.tensor_tensor(t2, mu_e, mu_e, op=Alu.mult)
                        pm.tensor_tensor(t3, mu_o, mu_o, op=Alu.mult)
                        pm.tensor_tensor(t2, t2, t3, op=Alu.add)
                        pm.tensor_tensor(t4, m2_e, m2_o, op=Alu.add)
                        nc.vector.scalar_tensor_tensor(
                            out=s1c, in0=t2, scalar=half, in1=t4,
                            op0=Alu.mult, op1=Alu.add)
                else:
                    c1 = cb[:, 2 * G:3 * G]
                    b1 = cb[:, 3 * G:4 * G]
                    nc.vector.tensor_scalar_mul(c1, rg, 1.0 + s)
                    nc.vector.tensor_scalar_mul(b1, bg, 1.0 + s)
                    nc.vector.tensor_scalar_mul(rg, rg, s)
                    nc.vector.tensor_scalar_mul(bg, bg, s)
                    ut = up.tile([P, G * N], dt_x, tag="u")
                    for g in range(G):
                        fs = slice(g * N, (g + 1) * N)
                        j = c * G + g
                        nc.scalar.activation(
                            out=ut[:, fs], in_=xt[:, fs], func=Act.Exp,
                            scale=rg[:, g:g + 1], bias=bg[:, g:g + 1],
                            accum_out=S2[:, j:j + 1])
                        nc.scalar.activation(
                            out=ut[:, fs], in_=xt[:, fs], func=Act.Exp,
                            scale=c1[:, g:g + 1], bias=b1[:, g:g + 1],
                            accum_out=S1[:, j:j + 1])

            # pos = m * S1 / S2 ; out = m > 0 ? pos : (m < 0 ? m : 0)
            recip(S2[:], S2[:])
            nc.vector.tensor_tensor(S1[:], S1[:], S2[:], op=Alu.mult)
            nc.vector.tensor_tensor(S1[:], S1[:], M[:], op=Alu.mult)
            # mask of m > 0 (uint8 — CopyPredicated needs an int mask)
            nc.vector.tensor_scalar(MK[:], M[:], 0.0, None, op0=Alu.is_gt)
            nc.vector.tensor_copy(R[:], M[:])
            nc.vector.copy_predicated(out=R[:], mask=MK[:], data=S1[:])

            # transpose R [128, T] -> RT so the store DMA has >=512B runs:
            # out row = t*128 + p ; RT[t_lo, k*128 + p] with t = k*128 + t_lo
            assert T % P == 0
            KB = T // P
            for k in range(KB):
                pt = psp.tile([P, P], f32, tag="pt")
                nc.tensor.transpose(pt[:], R[:, k * P:(k + 1) * P], ident[:])
                nc.vector.tensor_copy(RT[:, k * P:(k + 1) * P], pt[:])
            nc.sync.dma_start(
                out=out_d[:].rearrange("(k t p) -> t k p", k=KB, p=P),
                in_=RT[:].rearrange("t (k p) -> t k p", p=P))

    nc.compile()
    return nc


def _run(x: np.ndarray, scale: np.ndarray, trace: bool = False,
         build_kw: dict | None = None, **kw):
    from concourse.bass_utils import run_bass_kernel_spmd

    n_cores = 8
    B, Tm, X, Nn = x.shape          # 32, 256, 64, 256
    assert Nn == N
    rows = B * Tm * X
    rows_per_core = rows // n_cores
    s = float(np.asarray(scale))

    nc = _build(rows_per_core, s, **(build_kw or {}))
    xs = np.ascontiguousarray(np.asarray(x, dtype=np.float32)).reshape(
        n_cores, rows_per_core, N)
    in_maps = [{"x": xs[i]} for i in range(n_cores)]
    res = run_bass_kernel_spmd(nc, in_maps, list(range(n_cores)),
                               trace=trace, **kw)
    out = np.concatenate([r["out"].reshape(-1) for r in res.results], axis=0)
    return out.reshape(B, Tm, X).astype(np.float32), res


def kernel(x: np.ndarray, scale: np.ndarray) -> np.ndarray:
    return _run(x, scale)[0]


# revision 14
# speedup vs baseline: 1.0788x; 1.0788x over previous
"""Maxish pooling kernel for Trainium2 (8 NeuronCores, data-parallel).

Reference math (per row of length N):
    m  = max(x)
    rt = (x - m) / (m + 1e-8)
    pos = m * sum(exp(2*rt)) / sum(exp(rt))     # s == 1 softmax identity
    neg = m                                      # softmax sums to 1
    out = m > 0 ? pos : (m < 0 ? m : 0)

Layout: rows on partitions (128/tile), N=256 on the free axis.

Engine split (fast path, s == 1), x cast to bf16 during the DMA load
(SWDGE) so 16-bit DVE modes engage; rel-err budget (2e-2) dwarfs the
bf16 rounding:
  - DVE: pairwise tensor_tensor max folds at 2x rate (bf16), then a
    1x tensor_reduce on the folded remainder; per-tile bn_stats gives
    both sum(u) and sum(u^2).
  - ACT: per-tile exp (per-partition scale/bias APs). Optional z-path
    (z_tiles) and ACT-sum chunks (as_chunks) rebalance ACT vs DVE.
  - GpSimd: SWDGE descriptor generation + optional small/post ops
    (Pool TensorTensor only supports add/mult/sub, so no max folding
    there).
"""

import numpy as np

P = 128
N = 256
SMALL = 1e-8


def _build(n_rows: int, s: float, G: int = 16, fold_k: int = 3,
           z_dve: int = 0, z_gps: int = 8, as_chunks: int = 4,
           x_bufs: int = 4, u_bufs: int = 3, f_bufs: int = 3,
           xdt: str = "bf16", smalls_gpsimd: bool = True,
           post_gpsimd: bool = False, exact_recip: bool = False,
           look_ahead: int = 2):
    from concourse import bacc, mybir
    from concourse import masks
    from concourse.tile import TileContext

    f32 = mybir.dt.float32
    bf16 = mybir.dt.bfloat16
    dt_x = bf16 if xdt == "bf16" else f32
    Act = mybir.ActivationFunctionType
    Alu = mybir.AluOpType
    Ax = mybir.AxisListType

    assert n_rows % (P * G) == 0
    T = n_rows // P          # tiles of [128, N]
    C = T // G               # chunks of G tiles
    fast = (s == 1.0)
    assert 0 <= fold_k <= 4 and N % (1 << fold_k) == 0
    assert 0 <= z_dve + z_gps <= G
    assert 0 <= as_chunks <= C

    nc = bacc.Bacc("TRN2", target_bir_lowering=False, debug=False,
                   num_devices=8)
    x_d = nc.declare_dram_parameter("x", [n_rows, N], f32, isOutput=False)
    out_d = nc.declare_dram_parameter("out", [n_rows], f32, isOutput=True)

    def recip(dst, src):
        if exact_recip:
            nc.vector.reciprocal(dst, src)
        else:
            nc.vector.reciprocal_approx_fast(dst, src)

    with TileContext(nc) as tc:
        with (
            tc.tile_pool(name="xp", bufs=x_bufs) as xp,
            tc.tile_pool(name="up", bufs=u_bufs) as up,
            tc.tile_pool(name="fold", bufs=f_bufs) as fp,
            tc.tile_pool(name="stat", bufs=1) as statp,
            tc.tile_pool(name="consts", bufs=6) as cpool,
            tc.tile_pool(name="psum", bufs=2, space="PSUM") as psp,
        ):
            M = statp.tile([P, T], f32, tag="M")       # per-row max
            S1 = statp.tile([P, T], f32, tag="S1")     # sum exp(2 rt)
            S2 = statp.tile([P, T], f32, tag="S2")     # sum exp(rt)
            R = statp.tile([P, T], f32, tag="R")       # final per-row result
            RT = statp.tile([P, T], f32, tag="RT")     # transposed result
            MK = statp.tile([P, T], mybir.dt.uint8, tag="MK")  # m>0 mask

            ident = statp.tile([P, P], f32, tag="ident")
            masks.make_identity(nc, ident[:])

            sm = nc.gpsimd if smalls_gpsimd else nc.vector
            pm = nc.gpsimd if post_gpsimd else nc.vector

            # pre-issue loads `look_ahead` chunks early so SWDGE desc-gen
            # (in-order on the Pool queue) isn't stuck behind z ops that
            # wait on mid-chunk DVE results
            xts: list = [None] * C

            def load(ci):
                xt = xp.tile([P, G * N], dt_x, tag="x")
                src = x_d[ci * G * P:(ci + 1) * G * P, :].rearrange(
                    "(g p) n -> p g n", p=P)
                xdma = nc.gpsimd if dt_x == bf16 else nc.sync
                xdma.dma_start(
                    out=xt[:].rearrange("p (g n) -> p g n", n=N), in_=src)
                xts[ci] = xt

            for ci in range(min(look_ahead, C)):
                load(ci)

            # spread the ACT-sum chunks evenly so their ACT burst overlaps
            # bn work of neighbouring chunks instead of forming a tail
            as_every = C // as_chunks if as_chunks else 0
            as_set = {as_every // 2 + i * as_every for i in range(as_chunks)}

            for c in range(C):
                if c + look_ahead < C:
                    load(c + look_ahead)
                xt = xts[c]
                xts[c] = None

                # DVE pairwise max folds (2x rate on bf16), then reduce
                red_in = xt[:].rearrange("p (g n) -> p g n", n=N)
                w = N
                for k in range(fold_k):
                    w //= 2
                    ft = fp.tile([P, G * w], dt_x, tag=f"f{k}")
                    f3 = ft[:].rearrange("p (g n) -> p g n", n=w)
                    nc.vector.tensor_tensor(
                        out=f3, in0=red_in[:, :, 0:w],
                        in1=red_in[:, :, w:2 * w], op=Alu.max)
                    red_in = f3
                mg = M[:, c * G:(c + 1) * G]
                nc.vector.tensor_reduce(out=mg, in_=red_in, axis=Ax.X,
                                        op=Alu.max)

                # per-chunk consts in a versioned pool tile so ACT's reads
                # of chunk c don't serialize against DVE writing chunk c+1
                cb = cpool.tile([P, 6 * G], f32, tag="cb")
                rg = cb[:, 0:G]
                bg = cb[:, G:2 * G]
                # rg = 1 / (m + eps), clamped to >= 0 so the exponent
                # r*(x-m) stays <= 0 (m<0 rows are masked later; without
                # the clamp they can overflow exp)
                sm.tensor_scalar_add(rg, mg, SMALL)
                recip(rg, rg)
                sm.tensor_scalar_max(rg, rg, 0.0)
                # bg = (m * -1) * rg = -m/(m+eps)
                nc.vector.scalar_tensor_tensor(
                    out=bg, in0=mg, scalar=-1.0, in1=rg,
                    op0=Alu.mult, op1=Alu.mult)

                if fast:
                    act_sums = c in as_set
                    # as-chunks take their sums from per-tile ACT accum, so
                    # they use no z-path (z tiles would have no sums)
                    zc = 0 if act_sums else z_dve + z_gps
                    ut = up.tile([P, G * N], dt_x, tag="u")
                    # z-path tiles: affine z=r*x+b on DVE/GPS, one big exp
                    for g in range(zc):
                        fs = slice(g * N, (g + 1) * N)
                        zeng = nc.vector if g < z_dve else nc.gpsimd
                        zeng.tensor_scalar(
                            out=ut[:, fs], in0=xt[:, fs],
                            scalar1=rg[:, g:g + 1], scalar2=bg[:, g:g + 1],
                            op0=Alu.mult, op1=Alu.add)
                    if zc:
                        zs = slice(0, zc * N)
                        nc.scalar.activation(out=ut[:, zs], in_=ut[:, zs],
                                             func=Act.Exp)
                    # plain tiles: per-tile exp with per-partition scale/bias
                    for g in range(zc, G):
                        fs = slice(g * N, (g + 1) * N)
                        j = c * G + g
                        if act_sums:
                            nc.scalar.activation(
                                out=ut[:, fs], in_=xt[:, fs], func=Act.Exp,
                                scale=rg[:, g:g + 1], bias=bg[:, g:g + 1],
                                accum_out=S2[:, j:j + 1])
                            nc.scalar.activation(
                                out=ut[:, fs], in_=ut[:, fs], func=Act.Square,
                                accum_out=S1[:, j:j + 1])
                        else:
                            nc.scalar.activation(
                                out=ut[:, fs], in_=xt[:, fs], func=Act.Exp,
                                scale=rg[:, g:g + 1], bias=bg[:, g:g + 1])

                    if not act_sums:
                        # both sums via per-tile bn_stats over u:
                        # 6-tuple [n_e, mu_e, M2_e, n_o, mu_o, M2_o];
                        # S2 = 128*(mu_e+mu_o);
                        # S1 = M2_e+M2_o+128*(mu_e^2+mu_o^2)
                        bst = cpool.tile([P, G * 6], f32, tag="bst")
                        for g in range(G):
                            nc.vector.bn_stats(
                                out=bst[:, g * 6:(g + 1) * 6],
                                in_=ut[:, g * N:(g + 1) * N])
                        bsg = bst[:].rearrange("p (g s) -> p s g", s=6)
                        mu_e, m2_e = bsg[:, 1], bsg[:, 2]
                        mu_o, m2_o = bsg[:, 4], bsg[:, 5]
                        s2c = S2[:, c * G:(c + 1) * G]
                        s1c = S1[:, c * G:(c + 1) * G]
                        t1 = cb[:, 2 * G:3 * G]
                        t2 = cb[:, 3 * G:4 * G]
                        t3 = cb[:, 4 * G:5 * G]
                        t4 = cb[:, 5 * G:6 * G]
                        half = float(N // 2)
                        pm.tensor_tensor(t1, mu_e, mu_o, op=Alu.add)
                        nc.vector.tensor_scalar_mul(s2c, t1, half)
                        pm.tensor_tensor(t2, mu_e, mu_e, op=Alu.mult)
                        pm.tensor_tensor(t3, mu_o, mu_o, op=Alu.mult)
                        pm.tensor_tensor(t2, t2, t3, op=Alu.add)
                        pm.tensor_tensor(t4, m2_e, m2_o, op=Alu.add)
                        nc.vector.scalar_tensor_tensor(
                            out=s1c, in0=t2, scalar=half, in1=t4,
                            op0=Alu.mult, op1=Alu.add)
                else:
                    c1 = cb[:, 2 * G:3 * G]
                    b1 = cb[:, 3 * G:4 * G]
                    nc.vector.tensor_scalar_mul(c1, rg, 1.0 + s)
                    nc.vector.tensor_scalar_mul(b1, bg, 1.0 + s)
                    nc.vector.tensor_scalar_mul(rg, rg, s)
                    nc.vector.tensor_scalar_mul(bg, bg, s)
                    ut = up.tile([P, G * N], dt_x, tag="u")
                    for g in range(G):
                        fs = slice(g * N, (g + 1) * N)
                        j = c * G + g
                        nc.scalar.activation(
                            out=ut[:, fs], in_=xt[:, fs], func=Act.Exp,
                            scale=rg[:, g:g + 1], bias=bg[:, g:g + 1],
                            accum_out=S2[:, j:j + 1])
                        nc.scalar.activation(
                            out=ut[:, fs], in_=xt[:, fs], func=Act.Exp,
                            scale=c1[:, g:g + 1], bias=b1[:, g:g + 1],
                            accum_out=S1[:, j:j + 1])

            # pos = m * S1 / S2 ; out = m > 0 ? pos : (m < 0 ? m : 0)
            recip(S2[:], S2[:])
            nc.vector.tensor_tensor(S1[:], S1[:], S2[:], op=Alu.mult)
            nc.vector.tensor_tensor(S1[:], S1[:], M[:], op=Alu.mult)
            # mask of m > 0 (uint8 — CopyPredicated needs an int mask)
            nc.vector.tensor_scalar(MK[:], M[:], 0.0, None, op0=Alu.is_gt)
            nc.vector.tensor_copy(R[:], M[:])
            nc.vector.copy_predicated(out=R[:], mask=MK[:], data=S1[:])

            # transpose R [128, T] -> RT so the store DMA has >=512B runs:
            # out row = t*128 + p ; RT[t_lo, k*128 + p] with t = k*128 + t_lo
            assert T % P == 0
            KB = T // P
            for k in range(KB):
                pt = psp.tile([P, P], f32, tag="pt")
                nc.tensor.transpose(pt[:], R[:, k * P:(k + 1) * P], ident[:])
                nc.vector.tensor_copy(RT[:, k * P:(k + 1) * P], pt[:])
            nc.sync.dma_start(
                out=out_d[:].rearrange("(k t p) -> t k p", k=KB, p=P),
                in_=RT[:].rearrange("t (k p) -> t k p", p=P))

    nc.compile()
    return nc


def _run(x: np.ndarray, scale: np.ndarray, trace: bool = False,
         build_kw: dict | None = None, **kw):
    from concourse.bass_utils import run_bass_kernel_spmd

    n_cores = 8
    B, Tm, X, Nn = x.shape          # 32, 256, 64, 256
    assert Nn == N
    rows = B * Tm * X
    rows_per_core = rows // n_cores
    s = float(np.asarray(scale))

    nc = _build(rows_per_core, s, **(build_kw or {}))
    xs = np.ascontiguousarray(np.asarray(x, dtype=np.float32)).reshape(
        n_cores, rows_per_core, N)
    in_maps = [{"x": xs[i]} for i in range(n_cores)]
    res = run_bass_kernel_spmd(nc, in_maps, list(range(n_cores)),
                               trace=trace, **kw)
    out = np.concatenate([r["out"].reshape(-1) for r in res.results], axis=0)
    return out.reshape(B, Tm, X).astype(np.float32), res


def kernel(x: np.ndarray, scale: np.ndarray) -> np.ndarray:
    return _run(x, scale)[0]


# revision 15
# speedup vs baseline: 1.1869x; 1.1002x over previous
"""Maxish pooling kernel for Trainium2 (8 NeuronCores, data-parallel).

Reference math (per row of length N):
    m  = max(x)
    rt = (x - m) / (m + 1e-8)
    pos = m * sum(exp(2*rt)) / sum(exp(rt))     # s == 1 softmax identity
    neg = m                                      # softmax sums to 1
    out = m > 0 ? pos : (m < 0 ? m : 0)

Layout: rows on partitions (128/tile), N=256 on the free axis. x is cast
to bf16 during the DMA load (SWDGE) so 16-bit DVE modes engage; the
rel-err budget (2e-2) dwarfs the bf16 rounding.

Engine split (fast path, s == 1):
  - DVE: pairwise tensor_tensor max folds at 2x (bf16) + 1x reduce on
    the remainder; per-tile bn_stats -> both sum(u) and sum(u^2); the
    bn->S1/S2 fixup runs ONCE at the end on [128,T] strided views.
  - ACT: per-tile exp; `z_gps` tiles/chunk get their z=r*x+b affine on
    GpSimd and share one big chunked exp call. `as_chunks` chunks take
    sums from ACT accum (exp + square) instead of bn, trading DVE for
    ACT time.
  - GpSimd: SWDGE cast-DMA descriptor generation + the z affines.

Pipelining: loads are issued `look_ahead` chunks early (the Pool queue
is in-order; desc-gen must not sit behind z ops), plain exps are
emitted before the z-dependent big exp (ACT is in-order), and bn for
chunk c is emitted after chunk c+1's folds/smalls/exps so the DVE
never head-blocks waiting on fresh ACT output.
"""

import numpy as np

P = 128
N = 256
SMALL = 1e-8


def _build(n_rows: int, s: float, G: int = 16, fold_k: int = 3,
           z_dve: int = 0, z_gps: int = 8, as_chunks: int = 5,
           x_bufs: int = 4, u_bufs: int = 3, f_bufs: int = 3,
           xdt: str = "bf16", exact_recip: bool = False,
           look_ahead: int = 2):
    from concourse import bacc, mybir
    from concourse import masks
    from concourse.tile import TileContext

    f32 = mybir.dt.float32
    bf16 = mybir.dt.bfloat16
    dt_x = bf16 if xdt == "bf16" else f32
    Act = mybir.ActivationFunctionType
    Alu = mybir.AluOpType
    Ax = mybir.AxisListType

    assert n_rows % (P * G) == 0
    T = n_rows // P          # tiles of [128, N]
    C = T // G               # chunks of G tiles
    fast = (s == 1.0)
    assert 0 <= fold_k <= 4 and N % (1 << fold_k) == 0
    assert 0 <= z_dve + z_gps <= G
    assert 0 <= as_chunks <= C

    nc = bacc.Bacc("TRN2", target_bir_lowering=False, debug=False,
                   num_devices=8)
    x_d = nc.declare_dram_parameter("x", [n_rows, N], f32, isOutput=False)
    out_d = nc.declare_dram_parameter("out", [n_rows], f32, isOutput=True)

    def recip(dst, src):
        if exact_recip:
            nc.vector.reciprocal(dst, src)
        else:
            nc.vector.reciprocal_approx_fast(dst, src)

    with TileContext(nc) as tc:
        with (
            tc.tile_pool(name="xp", bufs=x_bufs) as xp,
            tc.tile_pool(name="up", bufs=u_bufs) as up,
            tc.tile_pool(name="fold", bufs=f_bufs) as fp,
            tc.tile_pool(name="stat", bufs=1) as statp,
            tc.tile_pool(name="consts", bufs=6) as cpool,
            tc.tile_pool(name="psum", bufs=2, space="PSUM") as psp,
        ):
            M = statp.tile([P, T], f32, tag="M")       # per-row max
            S1 = statp.tile([P, T], f32, tag="S1")     # sum exp(2 rt)
            S2 = statp.tile([P, T], f32, tag="S2")     # sum exp(rt)
            R = statp.tile([P, T], f32, tag="R")       # result / post scratch
            RT = statp.tile([P, T], f32, tag="RT")     # transposed / scratch
            T3 = statp.tile([P, T], f32, tag="T3")     # post scratch
            T4 = statp.tile([P, T], f32, tag="T4")     # post scratch
            MK = statp.tile([P, T], mybir.dt.uint8, tag="MK")  # m>0 mask
            BST = statp.tile([P, T * 6], f32, tag="BST")  # bn 6-tuples
            if as_chunks:
                S1A = statp.tile([P, as_chunks * G], f32, tag="S1A")
                S2A = statp.tile([P, as_chunks * G], f32, tag="S2A")

            ident = statp.tile([P, P], f32, tag="ident")
            masks.make_identity(nc, ident[:])

            # pre-issue loads so SWDGE desc-gen (in-order Pool queue)
            # isn't stuck behind z ops that wait on mid-chunk DVE results
            xts: list = [None] * C

            def load(ci):
                xt = xp.tile([P, G * N], dt_x, tag="x")
                src = x_d[ci * G * P:(ci + 1) * G * P, :].rearrange(
                    "(g p) n -> p g n", p=P)
                xdma = nc.gpsimd if dt_x == bf16 else nc.sync
                xdma.dma_start(
                    out=xt[:].rearrange("p (g n) -> p g n", n=N), in_=src)
                xts[ci] = xt

            for ci in range(min(look_ahead, C)):
                load(ci)

            # spread ACT-sum chunks so their ACT burst overlaps bn work
            as_every = C // as_chunks if as_chunks else 0
            as_list = sorted(as_every // 2 + i * as_every
                             for i in range(as_chunks))
            as_idx = {c: i for i, c in enumerate(as_list)}

            pending = None  # (ut of previous chunk, chunk index)

            for c in range(C):
                if c + look_ahead < C:
                    load(c + look_ahead)
                xt = xts[c]
                xts[c] = None

                # DVE pairwise max folds (2x on bf16), then 1x reduce
                red_in = xt[:].rearrange("p (g n) -> p g n", n=N)
                w = N
                for k in range(fold_k):
                    w //= 2
                    ft = fp.tile([P, G * w], dt_x, tag=f"f{k}")
                    f3 = ft[:].rearrange("p (g n) -> p g n", n=w)
                    nc.vector.tensor_tensor(
                        out=f3, in0=red_in[:, :, 0:w],
                        in1=red_in[:, :, w:2 * w], op=Alu.max)
                    red_in = f3
                mg = M[:, c * G:(c + 1) * G]
                nc.vector.tensor_reduce(out=mg, in_=red_in, axis=Ax.X,
                                        op=Alu.max)

                # rg = 1/(m+eps) clamped >= 0 (m<0 rows masked later;
                # without the clamp they can overflow exp); bg = -m*rg
                cb = cpool.tile([P, 4 * G], f32, tag="cb")
                rg = cb[:, 0:G]
                bg = cb[:, G:2 * G]
                nc.vector.tensor_scalar_add(rg, mg, SMALL)
                recip(rg, rg)
                nc.vector.tensor_scalar_max(rg, rg, 0.0)
                nc.vector.scalar_tensor_tensor(
                    out=bg, in0=mg, scalar=-1.0, in1=rg,
                    op0=Alu.mult, op1=Alu.mult)

                if fast:
                    act_sums = c in as_idx
                    zc = 0 if act_sums else z_dve + z_gps
                    ut = up.tile([P, G * N], dt_x, tag="u")
                    if act_sums:
                        ai = as_idx[c]
                        for g in range(G):
                            fs = slice(g * N, (g + 1) * N)
                            j = ai * G + g
                            nc.scalar.activation(
                                out=ut[:, fs], in_=xt[:, fs], func=Act.Exp,
                                scale=rg[:, g:g + 1], bias=bg[:, g:g + 1],
                                accum_out=S2A[:, j:j + 1])
                            nc.scalar.activation(
                                out=ut[:, fs], in_=ut[:, fs],
                                func=Act.Square,
                                accum_out=S1A[:, j:j + 1])
                    else:
                        # plain tiles first so the in-order ACT queue can
                        # run them while GPS computes the z affines
                        for g in range(zc, G):
                            fs = slice(g * N, (g + 1) * N)
                            nc.scalar.activation(
                                out=ut[:, fs], in_=xt[:, fs], func=Act.Exp,
                                scale=rg[:, g:g + 1], bias=bg[:, g:g + 1])
                        for g in range(zc):
                            fs = slice(g * N, (g + 1) * N)
                            zeng = nc.vector if g < z_dve else nc.gpsimd
                            zeng.tensor_scalar(
                                out=ut[:, fs], in0=xt[:, fs],
                                scalar1=rg[:, g:g + 1],
                                scalar2=bg[:, g:g + 1],
                                op0=Alu.mult, op1=Alu.add)
                        if zc:
                            zs = slice(0, zc * N)
                            nc.scalar.activation(out=ut[:, zs],
                                                 in_=ut[:, zs],
                                                 func=Act.Exp)

                    # deferred bn of the PREVIOUS chunk: by now its exps
                    # are long done, so the in-order DVE queue never
                    # blocks waiting on fresh ACT output
                    if pending is not None:
                        put, pc = pending
                        for g in range(G):
                            j = pc * G + g
                            nc.vector.bn_stats(
                                out=BST[:, j * 6:(j + 1) * 6],
                                in_=put[:, g * N:(g + 1) * N])
                    pending = None if act_sums else (ut, c)
                else:
                    c1 = cb[:, 2 * G:3 * G]
                    b1 = cb[:, 3 * G:4 * G]
                    nc.vector.tensor_scalar_mul(c1, rg, 1.0 + s)
                    nc.vector.tensor_scalar_mul(b1, bg, 1.0 + s)
                    nc.vector.tensor_scalar_mul(rg, rg, s)
                    nc.vector.tensor_scalar_mul(bg, bg, s)
                    ut = up.tile([P, G * N], dt_x, tag="u")
                    for g in range(G):
                        fs = slice(g * N, (g + 1) * N)
                        j = c * G + g
                        nc.scalar.activation(
                            out=ut[:, fs], in_=xt[:, fs], func=Act.Exp,
                            scale=rg[:, g:g + 1], bias=bg[:, g:g + 1],
                            accum_out=S2[:, j:j + 1])
                        nc.scalar.activation(
                            out=ut[:, fs], in_=xt[:, fs], func=Act.Exp,
                            scale=c1[:, g:g + 1], bias=b1[:, g:g + 1],
                            accum_out=S1[:, j:j + 1])

            if fast:
                if pending is not None:
                    put, pc = pending
                    for g in range(G):
                        j = pc * G + g
                        nc.vector.bn_stats(
                            out=BST[:, j * 6:(j + 1) * 6],
                            in_=put[:, g * N:(g + 1) * N])

                # one-shot bn fixup on [128, T] strided views:
                # 6-tuple per tile: [n_e, mu_e, M2_e, n_o, mu_o, M2_o]
                # S2 = 128*(mu_e+mu_o); S1 = M2_e+M2_o+128*(mu_e^2+mu_o^2)
                bs = BST[:].rearrange("p (t s) -> p s t", s=6)
                mu_e, m2_e = bs[:, 1], bs[:, 2]
                mu_o, m2_o = bs[:, 4], bs[:, 5]
                half = float(N // 2)
                t1, t2 = R[:], RT[:]
                nc.vector.tensor_tensor(t1, mu_e, mu_o, op=Alu.add)
                nc.vector.tensor_scalar_mul(S2[:], t1, half)
                nc.vector.tensor_tensor(t2, mu_e, mu_e, op=Alu.mult)
                nc.vector.tensor_tensor(T3[:], mu_o, mu_o, op=Alu.mult)
                nc.vector.tensor_tensor(t2, t2, T3[:], op=Alu.add)
                nc.vector.tensor_tensor(T4[:], m2_e, m2_o, op=Alu.add)
                nc.vector.scalar_tensor_tensor(
                    out=S1[:], in0=t2, scalar=half, in1=T4[:],
                    op0=Alu.mult, op1=Alu.add)
                # overwrite as-chunk columns with their ACT-accum sums
                for i, c in enumerate(as_list):
                    cs = slice(c * G, (c + 1) * G)
                    a = slice(i * G, (i + 1) * G)
                    nc.vector.tensor_copy(S1[:, cs], S1A[:, a])
                    nc.vector.tensor_copy(S2[:, cs], S2A[:, a])

            # pos = m * S1 / S2 ; out = m > 0 ? pos : (m < 0 ? m : 0)
            recip(S2[:], S2[:])
            nc.vector.tensor_tensor(S1[:], S1[:], S2[:], op=Alu.mult)
            nc.vector.tensor_tensor(S1[:], S1[:], M[:], op=Alu.mult)
            # mask of m > 0 (uint8 — CopyPredicated needs an int mask)
            nc.vector.tensor_scalar(MK[:], M[:], 0.0, None, op0=Alu.is_gt)
            nc.vector.tensor_copy(R[:], M[:])
            nc.vector.copy_predicated(out=R[:], mask=MK[:], data=S1[:])

            # transpose R [128, T] -> RT so the store DMA has >=512B runs:
            # out row = t*128 + p ; RT[t_lo, k*128 + p] with t = k*128 + t_lo
            assert T % P == 0
            KB = T // P
            for k in range(KB):
                pt = psp.tile([P, P], f32, tag="pt")
                nc.tensor.transpose(pt[:], R[:, k * P:(k + 1) * P], ident[:])
                nc.vector.tensor_copy(RT[:, k * P:(k + 1) * P], pt[:])
            nc.sync.dma_start(
                out=out_d[:].rearrange("(k t p) -> t k p", k=KB, p=P),
                in_=RT[:].rearrange("t (k p) -> t k p", p=P))

    nc.compile()
    return nc


def _run(x: np.ndarray, scale: np.ndarray, trace: bool = False,
         build_kw: dict | None = None, **kw):
    from concourse.bass_utils import run_bass_kernel_spmd

    n_cores = 8
    B, Tm, X, Nn = x.shape          # 32, 256, 64, 256
    assert Nn == N
    rows = B * Tm * X
    rows_per_core = rows // n_cores
    s = float(np.asarray(scale))

    nc = _build(rows_per_core, s, **(build_kw or {}))
    xs = np.ascontiguousarray(np.asarray(x, dtype=np.float32)).reshape(
        n_cores, rows_per_core, N)
    in_maps = [{"x": xs[i]} for i in range(n_cores)]
    res = run_bass_kernel_spmd(nc, in_maps, list(range(n_cores)),
                               trace=trace, **kw)
    out = np.concatenate([r["out"].reshape(-1) for r in res.results], axis=0)
    return out.reshape(B, Tm, X).astype(np.float32), res


def kernel(x: np.ndarray, scale: np.ndarray) -> np.ndarray:
    return _run(x, scale)[0]


# revision 16
# speedup vs baseline: 1.4070x; 1.1855x over previous
"""Maxish pooling kernel for Trainium2 (8 NeuronCores, data-parallel).

Reference math (per row of length N):
    m  = max(x)
    rt = (x - m) / (m + 1e-8)
    pos = m * sum(exp(2*rt)) / sum(exp(rt))     # s == 1 softmax identity
    neg = m                                      # softmax sums to 1
    out = m > 0 ? pos : (m < 0 ? m : 0)

Layout: rows on partitions (128/tile), N=256 on the free axis. x is cast
to bf16 during the DMA load (SWDGE) so 16-bit DVE modes engage; the
rel-err budget (2e-2) dwarfs the bf16 rounding.

Engine split (fast path, s == 1):
  - DVE: pairwise tensor_tensor max folds at 2x (bf16) + 1x reduce on
    the remainder; per-tile bn_stats -> both sum(u) and sum(u^2); the
    bn->S1/S2 fixup runs ONCE at the end on [128,T] strided views.
  - ACT: per-tile exp; `z_gps` tiles/chunk get their z=r*x+b affine on
    GpSimd and share one big chunked exp call. `as_chunks` chunks take
    sums from ACT accum (exp + square) instead of bn, trading DVE for
    ACT time.
  - GpSimd: SWDGE cast-DMA descriptor generation + the z affines.

Pipelining: loads are issued `look_ahead` chunks early (the Pool queue
is in-order; desc-gen must not sit behind z ops), plain exps are
emitted before the z-dependent big exp (ACT is in-order), and bn for
chunk c is emitted after chunk c+1's folds/smalls/exps so the DVE
never head-blocks waiting on fresh ACT output.
"""

import numpy as np

P = 128
N = 256
SMALL = 1e-8


def _build(n_rows: int, s: float, G: int = 16, fold_k: int = 3,
           z_dve: int = 0, z_gps: int = 7, as_tiles: int = 2,
           x_bufs: int = 4, u_bufs: int = 3, f_bufs: int = 3,
           xdt: str = "bf16", exact_recip: bool = False,
           look_ahead: int = 2):
    from concourse import bacc, mybir
    from concourse import masks
    from concourse.tile import TileContext

    f32 = mybir.dt.float32
    bf16 = mybir.dt.bfloat16
    dt_x = bf16 if xdt == "bf16" else f32
    Act = mybir.ActivationFunctionType
    Alu = mybir.AluOpType
    Ax = mybir.AxisListType

    assert n_rows % (P * G) == 0
    T = n_rows // P          # tiles of [128, N]
    C = T // G               # chunks of G tiles
    fast = (s == 1.0)
    assert 0 <= fold_k <= 4 and N % (1 << fold_k) == 0
    assert 0 <= z_dve + z_gps + as_tiles <= G

    nc = bacc.Bacc("TRN2", target_bir_lowering=False, debug=False,
                   num_devices=8)
    x_d = nc.declare_dram_parameter("x", [n_rows, N], f32, isOutput=False)
    out_d = nc.declare_dram_parameter("out", [n_rows], f32, isOutput=True)

    def recip(dst, src):
        if exact_recip:
            nc.vector.reciprocal(dst, src)
        else:
            nc.vector.reciprocal_approx_fast(dst, src)

    with TileContext(nc) as tc:
        with (
            tc.tile_pool(name="xp", bufs=x_bufs) as xp,
            tc.tile_pool(name="up", bufs=u_bufs) as up,
            tc.tile_pool(name="fold", bufs=f_bufs) as fp,
            tc.tile_pool(name="stat", bufs=1) as statp,
            tc.tile_pool(name="consts", bufs=6) as cpool,
            tc.tile_pool(name="psum", bufs=2, space="PSUM") as psp,
        ):
            M = statp.tile([P, T], f32, tag="M")       # per-row max
            S1 = statp.tile([P, T], f32, tag="S1")     # sum exp(2 rt)
            S2 = statp.tile([P, T], f32, tag="S2")     # sum exp(rt)
            R = statp.tile([P, T], f32, tag="R")       # result / post scratch
            RT = statp.tile([P, T], f32, tag="RT")     # transposed / scratch
            T3 = statp.tile([P, T], f32, tag="T3")     # post scratch
            T4 = statp.tile([P, T], f32, tag="T4")     # post scratch
            MK = statp.tile([P, T], mybir.dt.uint8, tag="MK")  # m>0 mask
            BST = statp.tile([P, T * 6], f32, tag="BST")  # bn 6-tuples
            if as_tiles:
                S1A = statp.tile([P, C * as_tiles], f32, tag="S1A")
                S2A = statp.tile([P, C * as_tiles], f32, tag="S2A")

            ident = statp.tile([P, P], f32, tag="ident")
            masks.make_identity(nc, ident[:])

            # pre-issue loads so SWDGE desc-gen (in-order Pool queue)
            # isn't stuck behind z ops that wait on mid-chunk DVE results
            xts: list = [None] * C

            def load(ci):
                xt = xp.tile([P, G * N], dt_x, tag="x")
                src = x_d[ci * G * P:(ci + 1) * G * P, :].rearrange(
                    "(g p) n -> p g n", p=P)
                xdma = nc.gpsimd if dt_x == bf16 else nc.sync
                xdma.dma_start(
                    out=xt[:].rearrange("p (g n) -> p g n", n=N), in_=src)
                xts[ci] = xt

            for ci in range(min(look_ahead, C)):
                load(ci)

            pending = None  # (ut of previous chunk, chunk index)

            for c in range(C):
                if c + look_ahead < C:
                    load(c + look_ahead)
                xt = xts[c]
                xts[c] = None

                # DVE pairwise max folds (2x on bf16), then 1x reduce
                red_in = xt[:].rearrange("p (g n) -> p g n", n=N)
                w = N
                for k in range(fold_k):
                    w //= 2
                    ft = fp.tile([P, G * w], dt_x, tag=f"f{k}")
                    f3 = ft[:].rearrange("p (g n) -> p g n", n=w)
                    nc.vector.tensor_tensor(
                        out=f3, in0=red_in[:, :, 0:w],
                        in1=red_in[:, :, w:2 * w], op=Alu.max)
                    red_in = f3
                mg = M[:, c * G:(c + 1) * G]
                nc.vector.tensor_reduce(out=mg, in_=red_in, axis=Ax.X,
                                        op=Alu.max)

                # rg = 1/(m+eps) clamped >= 0 (m<0 rows masked later;
                # without the clamp they can overflow exp); bg = -m*rg
                cb = cpool.tile([P, 4 * G], f32, tag="cb")
                rg = cb[:, 0:G]
                bg = cb[:, G:2 * G]
                nc.vector.tensor_scalar_add(rg, mg, SMALL)
                recip(rg, rg)
                nc.vector.tensor_scalar_max(rg, rg, 0.0)
                nc.vector.scalar_tensor_tensor(
                    out=bg, in0=mg, scalar=-1.0, in1=rg,
                    op0=Alu.mult, op1=Alu.mult)

                if fast:
                    zc = z_dve + z_gps
                    gb = G - as_tiles        # tiles covered by bn
                    ut = up.tile([P, G * N], dt_x, tag="u")
                    # plain tiles first so the in-order ACT queue can
                    # run them while GPS computes the z affines
                    for g in range(zc, gb):
                        fs = slice(g * N, (g + 1) * N)
                        nc.scalar.activation(
                            out=ut[:, fs], in_=xt[:, fs], func=Act.Exp,
                            scale=rg[:, g:g + 1], bias=bg[:, g:g + 1])
                    # ACT-sum tiles: sums via accum (exp then square)
                    for g in range(gb, G):
                        fs = slice(g * N, (g + 1) * N)
                        j = c * as_tiles + (g - gb)
                        nc.scalar.activation(
                            out=ut[:, fs], in_=xt[:, fs], func=Act.Exp,
                            scale=rg[:, g:g + 1], bias=bg[:, g:g + 1],
                            accum_out=S2A[:, j:j + 1])
                        nc.scalar.activation(
                            out=ut[:, fs], in_=ut[:, fs], func=Act.Square,
                            accum_out=S1A[:, j:j + 1])
                    for g in range(zc):
                        fs = slice(g * N, (g + 1) * N)
                        zeng = nc.vector if g < z_dve else nc.gpsimd
                        zeng.tensor_scalar(
                            out=ut[:, fs], in0=xt[:, fs],
                            scalar1=rg[:, g:g + 1],
                            scalar2=bg[:, g:g + 1],
                            op0=Alu.mult, op1=Alu.add)
                    if zc:
                        zs = slice(0, zc * N)
                        nc.scalar.activation(out=ut[:, zs], in_=ut[:, zs],
                                             func=Act.Exp)

                    # deferred bn of the PREVIOUS chunk: by now its exps
                    # are long done, so the in-order DVE queue never
                    # blocks waiting on fresh ACT output
                    if pending is not None:
                        put, pc = pending
                        for g in range(gb):
                            j = pc * G + g
                            nc.vector.bn_stats(
                                out=BST[:, j * 6:(j + 1) * 6],
                                in_=put[:, g * N:(g + 1) * N])
                    pending = (ut, c)
                else:
                    c1 = cb[:, 2 * G:3 * G]
                    b1 = cb[:, 3 * G:4 * G]
                    nc.vector.tensor_scalar_mul(c1, rg, 1.0 + s)
                    nc.vector.tensor_scalar_mul(b1, bg, 1.0 + s)
                    nc.vector.tensor_scalar_mul(rg, rg, s)
                    nc.vector.tensor_scalar_mul(bg, bg, s)
                    ut = up.tile([P, G * N], dt_x, tag="u")
                    for g in range(G):
                        fs = slice(g * N, (g + 1) * N)
                        j = c * G + g
                        nc.scalar.activation(
                            out=ut[:, fs], in_=xt[:, fs], func=Act.Exp,
                            scale=rg[:, g:g + 1], bias=bg[:, g:g + 1],
                            accum_out=S2[:, j:j + 1])
                        nc.scalar.activation(
                            out=ut[:, fs], in_=xt[:, fs], func=Act.Exp,
                            scale=c1[:, g:g + 1], bias=b1[:, g:g + 1],
                            accum_out=S1[:, j:j + 1])

            if fast:
                if pending is not None:
                    put, pc = pending
                    for g in range(G - as_tiles):
                        j = pc * G + g
                        nc.vector.bn_stats(
                            out=BST[:, j * 6:(j + 1) * 6],
                            in_=put[:, g * N:(g + 1) * N])

                # one-shot bn fixup on [128, T] strided views:
                # 6-tuple per tile: [n_e, mu_e, M2_e, n_o, mu_o, M2_o]
                # S2 = 128*(mu_e+mu_o); S1 = M2_e+M2_o+128*(mu_e^2+mu_o^2)
                bs = BST[:].rearrange("p (t s) -> p s t", s=6)
                mu_e, m2_e = bs[:, 1], bs[:, 2]
                mu_o, m2_o = bs[:, 4], bs[:, 5]
                half = float(N // 2)
                t1, t2 = R[:], RT[:]
                nc.vector.tensor_tensor(t1, mu_e, mu_o, op=Alu.add)
                nc.vector.tensor_scalar_mul(S2[:], t1, half)
                nc.vector.tensor_tensor(t2, mu_e, mu_e, op=Alu.mult)
                nc.vector.tensor_tensor(T3[:], mu_o, mu_o, op=Alu.mult)
                nc.vector.tensor_tensor(t2, t2, T3[:], op=Alu.add)
                nc.vector.tensor_tensor(T4[:], m2_e, m2_o, op=Alu.add)
                nc.vector.scalar_tensor_tensor(
                    out=S1[:], in0=t2, scalar=half, in1=T4[:],
                    op0=Alu.mult, op1=Alu.add)
                # overwrite the as-tile columns with their ACT-accum
                # sums (single strided copies)
                if as_tiles:
                    s1v = S1[:].rearrange("p (c g) -> p c g", g=G)[
                        :, :, G - as_tiles:G]
                    s2v = S2[:].rearrange("p (c g) -> p c g", g=G)[
                        :, :, G - as_tiles:G]
                    a3 = S1A[:].rearrange("p (c a) -> p c a", a=as_tiles)
                    b3 = S2A[:].rearrange("p (c a) -> p c a", a=as_tiles)
                    nc.vector.tensor_copy(s1v, a3)
                    nc.vector.tensor_copy(s2v, b3)

            # pos = m * S1 / S2 ; out = m > 0 ? pos : (m < 0 ? m : 0)
            recip(S2[:], S2[:])
            nc.vector.tensor_tensor(S1[:], S1[:], S2[:], op=Alu.mult)
            nc.vector.tensor_tensor(S1[:], S1[:], M[:], op=Alu.mult)
            # mask of m > 0 (uint8 — CopyPredicated needs an int mask)
            nc.vector.tensor_scalar(MK[:], M[:], 0.0, None, op0=Alu.is_gt)
            nc.vector.tensor_copy(R[:], M[:])
            nc.vector.copy_predicated(out=R[:], mask=MK[:], data=S1[:])

            # transpose R [128, T] -> RT so the store DMA has >=512B runs:
            # out row = t*128 + p ; RT[t_lo, k*128 + p] with t = k*128 + t_lo
            assert T % P == 0
            KB = T // P
            for k in range(KB):
                pt = psp.tile([P, P], f32, tag="pt")
                nc.tensor.transpose(pt[:], R[:, k * P:(k + 1) * P], ident[:])
                nc.vector.tensor_copy(RT[:, k * P:(k + 1) * P], pt[:])
            nc.sync.dma_start(
                out=out_d[:].rearrange("(k t p) -> t k p", k=KB, p=P),
                in_=RT[:].rearrange("t (k p) -> t k p", p=P))

    nc.compile()
    return nc


def _run(x: np.ndarray, scale: np.ndarray, trace: bool = False,
         build_kw: dict | None = None, **kw):
    from concourse.bass_utils import run_bass_kernel_spmd

    n_cores = 8
    B, Tm, X, Nn = x.shape          # 32, 256, 64, 256
    assert Nn == N
    rows = B * Tm * X
    rows_per_core = rows // n_cores
    s = float(np.asarray(scale))

    nc = _build(rows_per_core, s, **(build_kw or {}))
    xs = np.ascontiguousarray(np.asarray(x, dtype=np.float32)).reshape(
        n_cores, rows_per_core, N)
    in_maps = [{"x": xs[i]} for i in range(n_cores)]
    res = run_bass_kernel_spmd(nc, in_maps, list(range(n_cores)),
                               trace=trace, **kw)
    out = np.concatenate([r["out"].reshape(-1) for r in res.results], axis=0)
    return out.reshape(B, Tm, X).astype(np.float32), res


def kernel(x: np.ndarray, scale: np.ndarray) -> np.ndarray:
    return _run(x, scale)[0]
